# revision 18
# baseline (speedup 1.0000x reference)
"""CapsuleTransformConv on 8 Trainium2 NeuronCores.

Problem:  x [4,16,16,32,16] f32, matrix [288,16,512] f32.
          im2col (K=3, VALID) -> tile [4,14,14,288,16]
          votes  = einsum('bhwna,nac->bhwnc', tile, matrix)
          out    = votes.reshape(4,14,14,288,32,16)

Sharding: tensor-parallel over the filter*atom output axis (512 -> 64 per
core).  Every core reads the full x (2 MB) and its 64-wide slice of the
weights; writes its [784, 288, 64] slice of the output (~58 MB, the
dominant HBM traffic).

Per-core kernel:
  - x is loaded once into SBUF as 8 slabs of [128 rows, 512 (c,a)].
  - PE transposes produce xT [(c_in_octet, atom)=128 partitions,
    4 octets x 1024 (b,h,w)]; the 9 im2col taps are then just windowed
    (strided) access patterns over xT's free dim -- x is read from HBM
    exactly once.
  - Weights for 8 consecutive capsules (one c-octet of one tap) are laid
    out block-diagonally in a [128, 512] tile so one K=128 matmul computes
    8 independent [pos,16]@[16,64] capsule matmuls: out[pos, gc*64+f].
    The block-diagonal tile is built on-chip from a single compact 1.2 MB
    weight DMA (memset + 8 strided copies).
  - Main loop: 7 position windows (112 = 4b x 2i x 14j) x 9 taps; each
    iteration runs 4 matmuls (c-octets) into one 4-bank PSUM tile,
    one PSUM->SBUF copy (alternating Vector/Scalar engines), and one
    ~918 KB DMA to HBM (2 KB contiguous runs).
  - Matmuls run in float32r (TF32-class PE mode, 4x the fp32 rate);
    accumulation is fp32 in PSUM.
"""

import numpy as np

B, H, W, C, A = 4, 16, 16, 32, 16
KS = 3
OH = OW = 14
NCAP = KS * KS * C          # 288 capsules
FTOT = 512                  # filter*atom
NCORES = 8
FPC = FTOT // NCORES        # 64 output features per core
POS = B * OH * OW           # 784 output positions
NG = NCAP // 8              # 36 groups of 8 capsules = (tap, c-octet)

_NC_CACHE = {}
MM_MODE = "f32r"  # "f32" (exact, 4 cyc/row) or "f32r" (TF32-class, 1 cyc/row)


def _build_nc(mm_f32r=True):
    import concourse.bass as bass  # noqa: F401
    import concourse.mybir as mybir
    import concourse.tile as tile
    from concourse import bacc, masks

    f32 = mybir.dt.float32
    mmdt = mybir.dt.float32r if mm_f32r else mybir.dt.float32

    nc = bacc.Bacc(None, target_bir_lowering=False)
    x_d = nc.declare_dram_parameter("x", [B, H, W, C, A], f32, isOutput=False)
    m_d = nc.declare_dram_parameter("mat", [NCAP, A, FPC], f32, isOutput=False)
    o_d = nc.declare_dram_parameter("out", [POS, NCAP, FPC], f32, isOutput=True)

    x2d = x_d.rearrange("b h w c a -> (b h w) (c a)")   # [1024, 512]
    o4 = o_d.rearrange("(b q) n f -> b q n f", b=B)     # [4, 196, 288, 64]

    with tile.TileContext(nc) as tc:
        with (
            tc.tile_pool(name="const", bufs=1) as constp,
            tc.tile_pool(name="big", bufs=1) as bigp,
            tc.tile_pool(name="stage", bufs=4) as stagep,
            tc.tile_pool(name="tapp", bufs=2) as tapp,
            tc.tile_pool(name="psum", bufs=2, space="PSUM") as psump,
        ):
            ident = constp.tile([128, 128], f32, tag="ident")
            masks.make_identity(nc, ident[:])

            # ---- x: HBM -> SBUF once, as 8 row-slabs of [128, 512] ----
            x_sb = bigp.tile([128, 8 * 512], f32, tag="x_sb")
            for s in range(8):
                nc.sync.dma_start(
                    x_sb[:, s * 512:(s + 1) * 512], x2d[s * 128:(s + 1) * 128, :]
                )

            # ---- weights: block-diagonal wpack, built in chunks ----
            # wpack[(gc,a), g*512 + gc*64 + f] = matrix[g*8+gc, a, f], else 0.
            # FP32r matmul inputs must be produced by a rounding instruction
            # (never by DMA), so paint DMAs land in a transient f32 tile and
            # a full-partition engine copy rounds each chunk into wpack.
            wpack = bigp.tile([128, NG * 512], mmdt, tag="wpack")
            msrc = m_d.rearrange("(g gc) a f -> gc a g f", gc=8)
            GPC = 4  # groups per chunk
            for c in range(NG // GPC):
                wtmp = tapp.tile([128, GPC * 512], f32, tag="wtmp")
                nc.gpsimd.memset(wtmp[:], 0.0)
                wtv = wtmp[:].rearrange("p (g v) -> p g v", g=GPC)
                for gc in range(8):
                    nc.sync.dma_start(
                        wtv[gc * 16:(gc + 1) * 16, :, gc * FPC:(gc + 1) * FPC],
                        msrc[gc, :, c * GPC:(c + 1) * GPC, :],
                    )
                nc.vector.tensor_copy(
                    wpack[:, c * GPC * 512:(c + 1) * GPC * 512], wtmp[:]
                )

            # ---- xT: PE-transpose x into [(dc,a), oct*1024 + (b,h,w)] ----
            xt = bigp.tile([128, 4 * 1024], f32, tag="xt")
            for oct in range(4):
                for s in range(8):
                    tr = psump.tile([128, 128], f32, tag="mm")
                    nc.tensor.transpose(
                        tr[:],
                        x_sb[:, s * 512 + oct * 128: s * 512 + (oct + 1) * 128],
                        ident[:],
                    )
                    dst = xt[:, oct * 1024 + s * 128: oct * 1024 + (s + 1) * 128]
                    if s % 2 == 0:
                        nc.vector.tensor_copy(dst, tr[:])
                    else:
                        nc.scalar.copy(dst, tr[:])

            xtv = xt[:].rearrange("p (o b h w) -> p o b h w", o=4, b=B, h=H)

            # ---- main loop: 9 taps (outer) x per-batch pos windows ----
            # The matmul stationary operand must be a single flat free dim
            # (walrus constraint), so per tap we compact the im2col gather
            # into tap[(dc,a), oct*784 + (b,i,j)] with GPSIMD copies.
            it = 0
            for kk in range(9):
                ki, kj = kk // 3, kk % 3
                tap = tapp.tile([128, 4 * POS], mmdt, tag="tap")
                for oct in range(4):
                    dst = tap[:, oct * POS:(oct + 1) * POS].rearrange(
                        "p (b i j) -> p b i j", b=B, i=OH
                    )
                    nc.gpsimd.tensor_copy(
                        dst, xtv[:, oct, :, ki: ki + OH, kj: kj + OW]
                    )
                for b in range(B):
                    for i0, ni in ((0, 8), (8, 6)):
                        m = ni * OW  # 112 or 84 output positions
                        ps = psump.tile([128, 2048], f32, tag="mm")
                        for oct in range(4):
                            g = kk * 4 + oct
                            off = oct * POS + b * (OH * OW) + i0 * OW
                            nc.tensor.matmul(
                                ps[0:m, oct * 512:(oct + 1) * 512],
                                tap[:, off: off + m],
                                wpack[:, g * 512:(g + 1) * 512],
                                start=True,
                                stop=True,
                            )
                        st = stagep.tile([128, 2048], f32, tag="st")
                        if it % 2 == 0:
                            nc.vector.tensor_copy(st[0:m, :], ps[0:m, :])
                        else:
                            nc.scalar.copy(st[0:m, :], ps[0:m, :])
                        # Alternate the two HWDGE rings (SP / ACT) so output
                        # DMAs pipeline across both.
                        dma_eng = nc.sync if it % 2 == 0 else nc.scalar
                        dma_eng.dma_start(
                            o4[b, i0 * OW: i0 * OW + m, kk * 32:(kk + 1) * 32, :],
                            st[0:m, :].rearrange("p (n f) -> p n f", n=32),
                        )
                        it += 1

    nc.compile()
    return nc


def _get_nc():
    key = MM_MODE
    if key not in _NC_CACHE:
        _NC_CACHE[key] = _build_nc(mm_f32r=(MM_MODE == "f32r"))
    return _NC_CACHE[key]


def kernel(x, matrix):
    from concourse.bass_utils import run_bass_kernel_spmd

    x = np.ascontiguousarray(x, dtype=np.float32)
    matrix = np.ascontiguousarray(matrix, dtype=np.float32)
    nc = _get_nc()
    in_maps = [
        {
            "x": x,
            "mat": np.ascontiguousarray(matrix[:, :, c * FPC:(c + 1) * FPC]),
        }
        for c in range(NCORES)
    ]
    r = run_bass_kernel_spmd(nc, in_maps, list(range(NCORES)))
    parts = [r.results[c]["out"] for c in range(NCORES)]
    full = np.concatenate(parts, axis=-1)            # [784, 288, 512]
    return np.ascontiguousarray(
        full.reshape(B, OH, OW, NCAP, 32, 16).astype(np.float32)
    )
